# revision 17
# baseline (speedup 1.0000x reference)
"""Contextual-attention kernel for Trainium2, batch-parallel over 8 NeuronCores.

Per core (one image, feature [256,64,64], shared mask [128,128]):
  1. fp2 = zero-bordered feature halves [128, 68, 68] (f at [1:65,1:65]);
     fdpj[cc][j] = contiguous [34,32] col-windows (j=0..2) of the padded
     downsampled feature (f32r) -> gram operands need no per-strip staging;
     plane[cc][(a,b,tw)] = bf16 parity-plane col-windows of fp2 (GpSimd
     copies) -> deconv patch-bank transposes get single-run lhsT views.
  2. Gram S[q,p]: 18 accumulating matmuls per [128,512] tile, lhsT and rhs
     both strided fdpj views; scaled by 1/||patch|| (squares + box sums +
     one channel-sum matmul per half, f32r fast path).
  3. fuse conv 1 (diag +-1 row-major) and 2 (diag +-1 col-major with wrap)
     as PE shift-matrix matmuls accumulating in PSUM. Score tiles carry
     2 zero guard cols each side so every big matmul is a full aligned
     512-wide write (has_written gives correct edge behavior); fuse2's
     p-wrap slivers are aligned 32-wide matmuls from staged scratches.
     One DVE add per chunk-half evacuates PSUM.
  4. softmax in p-major layout, fully per-128-col-block pipelined:
     PE-transpose masked scores -> DVE reduce_max -> per-partition bias ->
     ScalarE exp with accum_out summing the denominator in the same op ->
     reciprocal -> transpose back through diag(rcp) stationaries (scales
     each p column) -> evacuate with per-q mask mult into A_pad.
  5. RW deconv banks built late into recycled score-tile slots (4
     transposes per PSUM bank, batched copies split DVE/ScalarE).
  6. deconv: 512 accumulating bf16 matmuls vs RW; h-halves ordered so each
     contiguous 32-row output block stores while the next computes.
"""
import sys

sys.path.insert(0, "/opt/trn_rl_repo")

import numpy as np

import concourse.bass as bass
import concourse.bacc as bacc_mod
import concourse.mybir as mybir
import concourse.tile as tile
from concourse.masks import make_identity
from concourse.bass_utils import run_bass_kernel_spmd

F32 = mybir.dt.float32
F32R = mybir.dt.float32r
BF16 = mybir.dt.bfloat16
AX = mybir.AxisListType
OP = mybir.AluOpType
ACT = mybir.ActivationFunctionType

N_CORES = 8
C, H, W = 256, 64, 64
SCALE = 10.0
EPS = 1e-4
G = 2  # guard columns each side of the 1024-wide score rows


def build_nc(gram_dt="f32r", dec_dt="bf16", reps=1, dbg=False):
    nc = bacc_mod.Bacc("TRN2", target_bir_lowering=False, debug=False)
    feat = nc.dram_tensor("feature", [C, H, W], F32, kind="ExternalInput")
    mask0 = nc.dram_tensor("mask0", [128, 128], F32, kind="ExternalInput")
    out_d = nc.dram_tensor("out", [C, H, W], F32, kind="ExternalOutput")
    dbg_d = None
    if dbg:
        dbg_d = {
            "d_m0": nc.dram_tensor("d_m0", [128, 8, 1028], F32, kind="ExternalOutput"),
            "d_m1": nc.dram_tensor("d_m1", [128, 8, 1028], F32, kind="ExternalOutput"),
            "d_m2": nc.dram_tensor("d_m2", [128, 8, 1028], F32, kind="ExternalOutput"),
            "d_ap": nc.dram_tensor("d_ap", [128, 8, 34, 34], BF16, kind="ExternalOutput"),
            "d_rn": nc.dram_tensor("d_rn", [128, 8], F32, kind="ExternalOutput"),
            "d_mm": nc.dram_tensor("d_mm", [128, 8], F32, kind="ExternalOutput"),
            "d_et": nc.dram_tensor("d_et", [128, 8, 1024], F32, kind="ExternalOutput"),
            "d_fj": nc.dram_tensor("d_fj", [128, 6, 34, 32], F32, kind="ExternalOutput"),
        }

    with tile.TileContext(nc) as tc:
        with (
            tc.tile_pool(name="big", bufs=3) as big,      # 3 x 32.9 KB
            tc.tile_pool(name="fpa", bufs=2) as fpa,      # 2 x 18.5 KB
            tc.tile_pool(name="fdj", bufs=6) as fdj,      # 6 x 4.25 KB
            tc.tile_pool(name="pln", bufs=8) as pln,      # 8 x 2.125 KB
            tc.tile_pool(name="med", bufs=1) as med,      # scratch 12.8 KB
            tc.tile_pool(name="etp", bufs=4) as etp,      # 2 x 2 KB
            tc.tile_pool(name="shr", bufs=2) as shr,      # raw shift staging
            tc.tile_pool(name="sml", bufs=1) as sml,
            tc.tile_pool(name="psp", bufs=8, space="PSUM") as psp,
        ):
            for rep in range(reps):
                _body(nc, tc, big, fpa, fdj, pln, med, etp, shr, sml,
                      psp, feat, mask0, out_d, rep, dbg_d)
    nc.finalize()
    return nc


def _make_shift(nc, t, delta):
    """t[k, q] = 1 iff k - q == delta (so (t.T @ M)[q, :] = M[q + delta, :])."""
    nc.gpsimd.memset(t, 0.0)
    nc.gpsimd.affine_select(
        out=t, in_=t, compare_op=OP.not_equal, fill=1.0,
        base=-delta, pattern=[[-1, 128]], channel_multiplier=1)


def _keep_rows(nc, t, lo, hi):
    """Zero partitions outside [lo, hi]."""
    nc.gpsimd.affine_select(
        out=t, in_=t, compare_op=OP.is_ge, fill=0.0,
        base=-lo, pattern=[[0, 128]], channel_multiplier=1)
    nc.gpsimd.affine_select(
        out=t, in_=t, compare_op=OP.is_ge, fill=0.0,
        base=hi, pattern=[[0, 128]], channel_multiplier=-1)


def _emit_group(nc, terms):
    n = len(terms)
    for i, (o, l, r) in enumerate(terms):
        nc.tensor.matmul(o, l, r, start=(i == 0), stop=(i == n - 1),
                         skip_group_check=True)


def _body(nc, tc, big, fpa, fdj, pln, med, etp, shr, sml, psp,
          feat, mask0, out_d, rep, dbg_d=None):
    DW = 1024 + 2 * G  # guarded score row width

    # ---------------- constants ----------------
    identf = shr.tile([128, 128], F32, tag="shraw", name=f"identf_{rep}")
    make_identity(nc, identf)
    ident_r = sml.tile([128, 128], F32R, tag="ident_r")
    nc.vector.tensor_copy(ident_r[:], identf[:])
    ident_bf = sml.tile([128, 128], BF16, tag="ident_bf")
    nc.vector.tensor_copy(ident_bf[:], identf[:])
    onesf = shr.tile([128, 128], F32, tag="ones_hold", name=f"onesf_{rep}")
    nc.any.memset(onesf[:], 1.0)
    ones_r = sml.tile([128, 128], F32R, tag="ones_r")
    nc.vector.tensor_copy(ones_r[:], onesf[:])
    zf = shr.tile([128, 128], F32, tag="shraw", name=f"zf_{rep}")
    nc.vector.memset(zf[:, 0:64], 0.0)
    zer = sml.tile([128, 64], F32R, tag="zer")
    nc.vector.tensor_copy(zer[:], zf[:, 0:64])

    shn = ("p1", "m1", "p32", "m32", "bp1", "bm1", "bp32", "bm32", "wp", "wm")
    shd = (1, -1, 32, -32, -127, 127, -96, 96, -95, 95)
    sh = {}
    for nm, d in zip(shn, shd):
        traw = shr.tile([128, 128], F32, tag="shraw", name=f"shr_{rep}_{nm}")
        _make_shift(nc, traw[:], d)
        if nm == "wp":
            _keep_rows(nc, traw[:], 1, 31)
        elif nm == "wm":
            _keep_rows(nc, traw[:], 96, 126)
        t = sml.tile([128, 128], F32R, tag=f"sh_{nm}", name=f"sh_{rep}_{nm}")
        nc.vector.tensor_copy(t[:], traw[:])
        sh[nm] = t
    slv = {}
    for nm in ("pm", "pb", "mm", "mb"):
        t = sml.tile([128, 32], F32R, tag=f"sl_{nm}", name=f"sl_{rep}_{nm}")
        nc.vector.tensor_copy(t[:], zer[:, 0:32])
        slv[nm] = t

    # ---------------- feature loads (first: contiguous, HWDGE-priority) ----
    # fp2[cc]: [128, 68, 64]; feature rows at 1..64 (full-width), row pads 0
    # and 65..67 zero. Column edges are handled by the staging copies.
    fp2 = []
    for cc in range(2):
        t = fpa.tile([128, 68, 64], F32, tag="fpa", name=f"fp2_{rep}_{cc}")
        nc.sync.dma_start(t[:, 1:65, :], feat[cc * 128:(cc + 1) * 128])
        nc.any.memset(t[:, 0:1, :], 0.0)
        nc.any.memset(t[:, 65:68, :], 0.0)
        fp2.append(t)

    # ---------------- shared scratch (mask then norm) ----------------
    scr = med.tile([128, 2180], F32, tag="scr", name=f"scr_{rep}")
    scf = scr[:]
    n2t = med.tile([128, 1024], F32R, tag="n2t", name=f"n2t_{rep}")

    # ---------------- feature loads (first: contiguous, HWDGE-priority) ----
    # fp2[cc]: [128, 68, 64]; feature rows at 1..64 (full-width), row pads 0
    # and 65..67 zero. Column edges are handled by the staging copies.
    fp2 = []
    for cc in range(2):
        t = fpa.tile([128, 68, 64], F32, tag="fpa", name=f"fp2_{rep}_{cc}")
        nc.sync.dma_start(t[:, 1:65, :], feat[cc * 128:(cc + 1) * 128])
        nc.any.memset(t[:, 0:1, :], 0.0)
        nc.any.memset(t[:, 65:68, :], 0.0)
        fp2.append(t)

    # ---------------- shared scratch (mask then norm) ----------------
    scr = med.tile([128, 2180], F32, tag="scr", name=f"scr_{rep}")
    scf = scr[:]
    n2t = med.tile([128, 1024], F32R, tag="n2t", name=f"n2t_{rep}")

    # ---------------- mask -> mm_q [128, 8] ----------------
    for k, (dy, dx) in enumerate(((0, 0), (0, 1), (1, 0), (1, 1))):
        off = 0 if k == 0 else 1024
        dst = scf[0:1, off:off + 1024].rearrange("o (a b) -> o a b", a=32)
        nc.sync.dma_start(dst, mask0[dy::4, dx::4][None])
        if k > 0:
            nc.vector.tensor_add(scf[0:1, 0:1024], scf[0:1, 0:1024],
                                 scf[0:1, 1024:2048])
    msum = scf[0:1, 0:1024].rearrange("o (a b) -> o a b", a=32)
    mdp = scf[0:1, 1024:2180].rearrange("o (a b) -> o a b", a=34)
    mbx = scf[0:1, 0:1088].rearrange("o (a b) -> o a b", a=34)
    nc.any.memset(mdp[:], 0.0)
    nc.vector.tensor_scalar(mdp[:, 1:33, 1:33], msum[:], 2.5, None, OP.is_ge)
    nc.vector.tensor_add(mbx[:], mdp[:, :, 0:32], mdp[:, :, 1:33])
    nc.vector.tensor_add(mbx[:], mbx[:], mdp[:, :, 2:34])
    mbox = scf[0:1, 1088:2112].rearrange("o (a b) -> o a b", a=32)
    nc.vector.tensor_add(mbox[:], mbx[:, 0:32, :], mbx[:, 1:33, :])
    nc.vector.tensor_add(mbox[:], mbox[:], mbx[:, 2:34, :])
    mmrow = scf[0:1, 0:1024]
    nc.vector.tensor_scalar(mmrow[:].rearrange("o (a b) -> o a b", a=32),
                            mbox[:], 0.0, None, OP.is_equal)
    mm_q = sml.tile([128, 8], F32, tag="mm_q")
    for c8 in range(8):
        nc.sync.dma_start(mm_q[:, c8:c8 + 1], mmrow[:, 128 * c8:128 * (c8 + 1)])

    # fdpj[cc][j]: [128, 34, 32] f32r; global padded-downsample col x at
    # local x-j; interior value fdp[y, x] = fp2[2y-1, 2x-1] (y,x in 1..32).
    fdpj = []
    for cc in range(2):
        row = []
        for j in range(3):
            t = fdj.tile([128, 34, 32], F32R, tag="fdj",
                         name=f"fdpj_{rep}_{cc}_{j}")
            nc.vector.tensor_copy(t[:, 0:1, :], zer[:, 0:32].unsqueeze(1))
            nc.vector.tensor_copy(t[:, 33:34, :], zer[:, 0:32].unsqueeze(1))
            x_lo, x_hi = max(1, j), min(32, j + 31)
            if j == 0:
                nc.vector.tensor_copy(t[:, 1:33, 0:1], zer[:, 0:32].unsqueeze(2))
            if j == 2:
                nc.vector.tensor_copy(t[:, 1:33, 31:32], zer[:, 0:32].unsqueeze(2))
            nc.vector.tensor_copy(
                t[:, 1:33, x_lo - j:x_hi - j + 1],
                fp2[cc][:, 1:65:2, 2 * x_lo - 2:2 * x_hi - 1:2])
            row.append(t)
        fdpj.append(row)

    # plane[cc][(a,b,tw)][i, l] = fp2[cc][2i+a, 2(tw+l)+b]  (bf16, GpSimd)
    planes = []
    for cc in range(2):
        d = {}
        for a in range(2):
            for b in range(2):
                for tw in range(2):
                    t = pln.tile([128, 34, 32], BF16, tag="pln",
                                 name=f"pl_{rep}_{cc}_{a}_{b}_{tw}")
                    c0 = 2 * tw + b - 1
                    if c0 < 0:
                        nc.gpsimd.memset(t[:, :, 0:1], 0.0)
                        nc.gpsimd.tensor_copy(
                            t[:, :, 1:32], fp2[cc][:, a:68:2, 1:62:2])
                    elif c0 + 62 > 63:
                        nc.gpsimd.memset(t[:, :, 31:32], 0.0)
                        nc.gpsimd.tensor_copy(
                            t[:, :, 0:31], fp2[cc][:, a:68:2, c0:c0 + 61:2])
                    else:
                        nc.gpsimd.tensor_copy(
                            t[:], fp2[cc][:, a:68:2, c0:c0 + 63:2])
                    d[(a, b, tw)] = t
        planes.append(d)

    # ---------------- rnorm path (into scr, after mask consumed) --------
    sq1 = scf[:, 0:1088].rearrange("p (a b) -> p a b", a=34)
    nbx = scf[:, 1088:2176].rearrange("p (a b) -> p a b", a=34)
    sq2 = scf[:, 1088:2112]
    nc.scalar.square(sq1[:], fdpj[0][1][:])
    nc.scalar.square(sq2[:, 0:1024].rearrange("p (a b) -> p a b", a=32),
                     fdpj[1][1][:, 1:33, :])
    nc.vector.tensor_add(sq1[:, 1:33, :], sq1[:, 1:33, :],
                         sq2[:, 0:1024].rearrange("p (a b) -> p a b", a=32))
    nc.vector.tensor_copy(nbx[:], sq1[:])
    nc.vector.tensor_add(nbx[:, :, 1:32], nbx[:, :, 1:32], sq1[:, :, 0:31])
    nc.vector.tensor_add(nbx[:, :, 0:31], nbx[:, :, 0:31], sq1[:, :, 1:32])
    n2 = n2t[:].rearrange("p (a b) -> p a b", a=32)
    nc.vector.tensor_add(n2[:], nbx[:, 0:32, :], nbx[:, 1:33, :])
    nc.vector.tensor_add(n2[:], n2[:], nbx[:, 2:34, :])
    nrm = scf[:, 0:1024]
    rnorm_q = sml.tile([128, 8], F32, tag="rnorm_q")

    def _emit_norm_matmuls():
        for hh in range(2):
            ps = psp.tile([128, 512], F32, tag="ps", name=f"nps_{rep}_{hh}")
            nc.tensor.matmul(ps[:], ones_r[:], n2t[:, 512 * hh:512 * hh + 512],
                             start=True, stop=True)
            nc.scalar.sqrt(nrm[:, 512 * hh:512 * hh + 512], ps[:])
        nc.vector.tensor_scalar_max(nrm[:], nrm[:], EPS)
        nc.vector.reciprocal(nrm[:], nrm[:])
        # scatter the 1024-wide row into [128, 8] via tiny PE outer products
        psn = psp.tile([128, 16], F32, tag="ps", name=f"rnq_{rep}")
        for c8 in range(8):
            nc.tensor.matmul(psn[:, 2 * c8:2 * c8 + 2],
                             nrm[0:1, 128 * c8:128 * (c8 + 1)],
                             onesf[0:1, 0:2], start=True, stop=True)
        nc.vector.tensor_copy(rnorm_q[:], psn[:, 0:16:2])

    # ---------------- Gram -> M0 (guarded [128, 8, DW]) ----------------
    M0 = big.tile([128, 8, DW], F32R, tag="big", name=f"m0_{rep}")
    zb = zer[:, 0:G].unsqueeze(1).to_broadcast([128, 8, G])
    nc.vector.tensor_copy(M0[:, :, 0:G], zb)
    nc.vector.tensor_copy(M0[:, :, G + 1024:], zb)
    shifts = [(i, j) for i in range(3) for j in range(3)]

    def _gram_evac(t, hh, ps):
        nc.vector.tensor_scalar_mul(
            M0[:, t, G + 512 * hh:G + 512 * hh + 512], ps[:],
            rnorm_q[:, t:t + 1])

    pending = []
    for t in range(8):
        if t == 2:
            # norm matmuls sit here on the PE queue so tiles 0/1 overlap the
            # load+boxsum chain; their evacs (DVE) must follow the rnorm_q
            # producer, hence the deferred flush.
            _emit_norm_matmuls()
            for args in pending:
                _gram_evac(*args)
            pending = []
        for hh in range(2):
            ps = psp.tile([128, 512], F32, tag="ps", name=f"gps_{rep}_{t}_{hh}")
            k = 0
            for cc in range(2):
                for (i, j) in shifts:
                    lhsT = fdpj[cc][j][:, i + 4 * t:i + 4 * t + 4, :]
                    rhs = fdpj[cc][j][:, i + 16 * hh:i + 16 * hh + 16, :]
                    nc.tensor.matmul(ps[:], lhsT, rhs,
                                     start=(k == 0), stop=(k == 17))
                    k += 1
            if t < 2:
                pending.append((t, hh, ps))
            else:
                _gram_evac(t, hh, ps)

    if dbg_d is not None:
        nc.sync.dma_start(dbg_d["d_m0"][:], M0[:].bitcast(F32))
        for _cc in range(2):
            for _j in range(3):
                nc.sync.dma_start(dbg_d["d_fj"][:, 3 * _cc + _j],
                                  fdpj[_cc][_j][:].bitcast(F32))
        nc.sync.dma_start(dbg_d["d_rn"][:], rnorm_q[:])
        nc.sync.dma_start(dbg_d["d_mm"][:], mm_q[:])


    # ---------------- fuse1 (diag +-1, row-major) ----------------
    M1 = big.tile([128, 8, DW], F32R, tag="big", name=f"m1_{rep}")
    nc.vector.tensor_copy(M1[:, :, 0:G], zb)
    nc.vector.tensor_copy(M1[:, :, G + 1024:], zb)
    for ch in range(8):
        for hh in range(2):
            ps = psp.tile([128, 512], F32, tag="ps", name=f"f1_{rep}_{ch}_{hh}")
            src = slice(G + 1 + 512 * hh, G + 513 + 512 * hh)
            srcm = slice(G - 1 + 512 * hh, G + 511 + 512 * hh)
            terms = [(ps[:], sh["p1"][:], M0[:, ch, src])]
            if ch < 7:
                terms.append((ps[:], sh["bp1"][:], M0[:, ch + 1, src]))
            terms.append((ps[:], sh["m1"][:], M0[:, ch, srcm]))
            if ch > 0:
                terms.append((ps[:], sh["bm1"][:], M0[:, ch - 1, srcm]))
            _emit_group(nc, terms)
            nc.vector.tensor_add(M1[:, ch, G + 512 * hh:G + 512 * hh + 512],
                                 M0[:, ch, G + 512 * hh:G + 512 * hh + 512],
                                 ps[:])

    if dbg_d is not None:
        nc.sync.dma_start(dbg_d["d_m1"][:], M1[:].bitcast(F32))

    # ---------------- fuse2 (diag +-1, col-major w/ wrap) ----------------
    M2 = big.tile([128, 8, DW], F32R, tag="big", name=f"m2_{rep}")
    nc.vector.tensor_copy(M2[:, :, 0:G], zb)
    nc.vector.tensor_copy(M2[:, :, G + 1024:], zb)
    for ch in range(8):
        up = sh["bp32"] if ch < 7 else sh["wp"]
        up_src = ch + 1 if ch < 7 else 0
        dn = sh["bm32"] if ch > 0 else sh["wm"]
        dn_src = ch - 1 if ch > 0 else 7
        nc.vector.tensor_copy(slv["pm"][:, 0:31], M1[:, ch, G + 1:G + 32])
        nc.vector.tensor_copy(slv["pb"][:, 0:31], M1[:, up_src, G + 1:G + 32])
        nc.vector.tensor_copy(slv["mm"][:, 1:32], M1[:, ch, G + 992:G + 1023])
        nc.vector.tensor_copy(slv["mb"][:, 1:32], M1[:, dn_src, G + 992:G + 1023])
        for hh in range(2):
            ps = psp.tile([128, 512], F32, tag="ps", name=f"f2_{rep}_{ch}_{hh}")
            if hh == 0:
                sp = slice(G + 32, G + 544)
                terms = [
                    (ps[:], sh["p32"][:], M1[:, ch, sp]),
                    (ps[:], up[:], M1[:, up_src, sp]),
                    (ps[:, 32:512], sh["m32"][:], M1[:, ch, G:G + 480]),
                    (ps[:, 32:512], dn[:], M1[:, dn_src, G:G + 480]),
                    (ps[:, 0:32], sh["m32"][:], slv["mm"][:]),
                    (ps[:, 0:32], dn[:], slv["mb"][:]),
                ]
            else:
                sm = slice(G + 480, G + 992)
                terms = [
                    (ps[:], sh["m32"][:], M1[:, ch, sm]),
                    (ps[:], dn[:], M1[:, dn_src, sm]),
                    (ps[:, 0:480], sh["p32"][:], M1[:, ch, G + 544:G + 1024]),
                    (ps[:, 0:480], up[:], M1[:, up_src, G + 544:G + 1024]),
                    (ps[:, 480:512], sh["p32"][:], slv["pm"][:]),
                    (ps[:, 480:512], up[:], slv["pb"][:]),
                ]
            _emit_group(nc, terms)
            nc.vector.tensor_add(M2[:, ch, G + 512 * hh:G + 512 * hh + 512],
                                 M1[:, ch, G + 512 * hh:G + 512 * hh + 512],
                                 ps[:])

    if dbg_d is not None:
        nc.sync.dma_start(dbg_d["d_m2"][:], M2[:].bitcast(F32))

    # ---------------- mask rows ----------------
    for t in range(8):
        nc.vector.tensor_scalar_mul(M2[:, t, G:G + 1024], M2[:, t, G:G + 1024],
                                    mm_q[:, t:t + 1])

    # ---------------- RW deconv banks (recycle M0's slot) ----------------
    RW = []
    for cc in range(2):
        rw = big.tile([128, 8, 16, 128], BF16, tag="big", name=f"rw_{rep}_{cc}")
        for u in range(4):
            for v in range(4):
                a, s = u % 2, u // 2
                b, tw = v % 2, v // 2
                pl_t = planes[cc][(a, b, tw)]
                for gq in range(2):
                    ps = psp.tile([128, 512], BF16, tag="ps",
                                  name=f"rwt_{rep}_{cc}_{u}_{v}_{gq}")
                    for k in range(4):
                        qc = 4 * gq + k
                        nc.tensor.transpose(
                            ps[:, 128 * k:128 * (k + 1)],
                            pl_t[:, s + 4 * qc:s + 4 * qc + 4, :],
                            ident_bf[:])
                    dst = rw[:, 4 * gq:4 * gq + 4, 4 * u + v, :]
                    src = ps[:].rearrange("p (k c) -> p k c", k=4)
                    if (u + v) % 2 == 0:
                        nc.vector.tensor_copy(dst, src)
                    else:
                        nc.scalar.copy(dst, src)
        RW.append(rw)

    # ---------------- p-major softmax -> A_pad ----------------
    A_pad = fpa.tile([128, 8, 34, 34], BF16, tag="fpa", name=f"ap_{rep}")
    nc.any.memset(A_pad[:, :, 0:1, :], 0.0)
    nc.any.memset(A_pad[:, :, 33:34, :], 0.0)
    nc.any.memset(A_pad[:, :, 1:33, 0:1], 0.0)
    nc.any.memset(A_pad[:, :, 1:33, 33:34], 0.0)
    mx8 = sml.tile([128, 8, 2], F32, tag="mx8")
    bias8 = sml.tile([128, 8], F32, tag="bias8")
    den8 = sml.tile([128, 8, 2], F32, tag="den8")
    rcp8 = sml.tile([128, 8], F32, tag="rcp8")
    diag = sml.tile([128, 8, 128], BF16, tag="diag")
    LAG = 3  # transpose-back trails the max/exp chain by LAG p-blocks
    Ets = {}

    def _sm_front(pt):
        Et = etp.tile([128, 1024], BF16, tag="et", name=f"et_{rep}_{pt}")
        Ets[pt] = Et
        pss = []
        for gq in range(2):
            ps = psp.tile([128, 512], F32R, tag="ps", name=f"mt_{rep}_{pt}_{gq}")
            for k in range(4):
                t = 4 * gq + k
                nc.tensor.transpose(ps[:, 128 * k:128 * (k + 1)],
                                    M2[:, t, G + 128 * pt:G + 128 * pt + 128],
                                    ident_r[:])
            nc.vector.reduce_max(mx8[:, pt, gq:gq + 1], ps[:].bitcast(F32),
                                 axis=AX.X)
            pss.append(ps)
        nc.vector.tensor_tensor(bias8[:, pt:pt + 1], mx8[:, pt, 0:1],
                                mx8[:, pt, 1:2], OP.max)
        nc.vector.tensor_scalar_mul(bias8[:, pt:pt + 1], bias8[:, pt:pt + 1],
                                    -SCALE)
        for gq in range(2):
            nc.scalar.activation(
                Et[:, 512 * gq:512 * gq + 512], pss[gq][:].bitcast(F32),
                ACT.Exp, bias=bias8[:, pt:pt + 1], scale=SCALE,
                accum_out=den8[:, pt, gq:gq + 1])
        nc.vector.tensor_add(rcp8[:, pt:pt + 1], den8[:, pt, 0:1],
                             den8[:, pt, 1:2])
        nc.vector.reciprocal(rcp8[:, pt:pt + 1], rcp8[:, pt:pt + 1])
        nc.vector.tensor_scalar_mul(diag[:, pt, :], ident_bf[:],
                                    rcp8[:, pt:pt + 1])

    def _sm_back(pt):
        Et = Ets.pop(pt)
        for tg in range(2):
            ps = psp.tile([128, 512], F32, tag="ps", name=f"eq_{rep}_{pt}_{tg}")
            for k in range(4):
                t = 4 * tg + k
                nc.tensor.matmul(ps[:, 128 * k:128 * (k + 1)],
                                 Et[:, 128 * t:128 * t + 128],
                                 diag[:, pt, :],
                                 start=True, stop=True)
            for k in range(4):
                t = 4 * tg + k
                dst = A_pad[:, t, 1 + 4 * pt:5 + 4 * pt, 1:33]
                src = ps[:, 128 * k:128 * (k + 1)].rearrange(
                    "p (a b) -> p a b", a=4)
                if k % 2 == 0:
                    nc.vector.tensor_scalar_mul(dst, src, mm_q[:, t:t + 1])
                else:
                    nc.scalar.mul(dst, src, mm_q[:, t:t + 1])

    for pt in range(8 + LAG):
        if pt < 8:
            _sm_front(pt)
        if pt >= LAG:
            _sm_back(pt - LAG)

    if dbg_d is not None:
        nc.sync.dma_start(dbg_d["d_ap"][:], A_pad[:])
        for _pt in range(8):
            pass

    # ---------------- deconv ----------------
    for cc in range(2):
        out_sb = big.tile([128, 64, 64], F32, tag="big", name=f"os_{rep}_{cc}")
        for hh in range(2):
            accs, cnt = {}, {}
            for ry in range(2):
                for rx in range(2):
                    accs[(ry, rx)] = psp.tile(
                        [128, 512], F32, tag="ps",
                        name=f"da_{rep}_{cc}_{hh}_{ry}_{rx}")
                    cnt[(ry, rx)] = 0
            for qc in range(8):
                for ry in range(2):
                    us = [u for u in range(4) if (u + 1) % 2 == ry]
                    for rx in range(2):
                        vs = [v for v in range(4) if (v + 1) % 2 == rx]
                        for u in us:
                            for v in vs:
                                sy = (ry + 1 - u) // 2
                                sx = (rx + 1 - v) // 2
                                rhs = A_pad[:, qc,
                                            1 + sy + 16 * hh:17 + sy + 16 * hh,
                                            1 + sx:33 + sx]
                                k = cnt[(ry, rx)]
                                nc.tensor.matmul(accs[(ry, rx)][:],
                                                 RW[cc][:, qc, 4 * u + v, :],
                                                 rhs, start=(k == 0),
                                                 stop=(k == 31))
                                cnt[(ry, rx)] += 1
            for ry in range(2):
                for rx in range(2):
                    dst = out_sb[:, 32 * hh + ry:32 * (hh + 1):2, rx::2]
                    nc.scalar.mul(dst, accs[(ry, rx)][:], 0.25)
            nc.sync.dma_start(
                out_d[cc * 128:(cc + 1) * 128, 32 * hh:32 * (hh + 1), :],
                out_sb[:, 32 * hh:32 * (hh + 1), :])


_NC_CACHE = {}


def _get_nc(cfg=("f32r", "bf16")):
    if cfg not in _NC_CACHE:
        _NC_CACHE[cfg] = build_nc(*cfg)
    return _NC_CACHE[cfg]


def kernel(feature: np.ndarray, mask: np.ndarray) -> np.ndarray:
    feature = np.ascontiguousarray(np.asarray(feature, dtype=np.float32))
    mask = np.asarray(mask, dtype=np.float32)
    nc = _get_nc()
    m0 = np.ascontiguousarray(mask[0, 0])
    in_maps = [{"feature": np.ascontiguousarray(feature[i]), "mask0": m0}
               for i in range(N_CORES)]
    res = run_bass_kernel_spmd(nc, in_maps, list(range(N_CORES)))
    return np.stack([np.asarray(res.results[i]["out"], dtype=np.float32)
                     for i in range(N_CORES)])
